# revision 31
# baseline (speedup 1.0000x reference)
"""Trainium2 Bass kernel for nn_ExpertMLP (MoE routing).

Strategy (expert-parallel, host-side dispatch):
  - E == n_cores == 8: core e owns expert e's weights.
  - Host computes the routing (which tokens hit expert e, with combined
    gate weight summed over duplicate top-k hits), gathers those tokens
    into a padded [C, H] buffer per expert, and ships core e:
        xt  = gathered tokens, transposed  [H, C]   (bf16)
        w1t = w1[e].T                      [H, F]   (bf16)
        w2t = w2[e].T                      [F, H]   (bf16)
        wc  = combined gate weights        [C]      (f32)
  - Device computes  y_e = (silu(x_e @ w1[e].T) @ w2[e].T) * wc[:, None]
    with all matmuls in bf16 (f32 PSUM accumulation).
  - Host scatter-adds per-expert outputs back into the full [S, H] output.

Device kernel layout (per core):
  Phase 1 computes h in F-major layout (hT [F, C]) so that phase 2 can use
  hT tiles directly as the stationary matmul operand -- no transposes
  anywhere on device (host provides x/w1/w2 pre-transposed).

  Per 512-token chunk:
    phase 1:  for f in 32:  psum_h[128, cw] += w1t[h, f-tile].T @ xt[h]   (8 MMs)
              silu(psum_h) -> hs[:, f, :]  (bf16, ACT engine)
    pass A (interleaved, shifted by two f): y[:, 0:512] accumulated over f
              psum_yA[t] += hs[f, t-tile].T @ w2t[f][:, 0:512]
    pass B:   y[:, 512:1024] accumulated over f (re-reads hs), t-outer
    epilogue: y_sbuf = psum_y * wc  (per-partition scalar), DMA out 4-way.

  DMAs are emitted in consumption order (w1 column groups interleaved with
  the w2 tiles used alongside them; x prefetched one chunk ahead) and the
  PE is pre-warmed with dependency-free matmuls so the HAM clock gate is
  at 8/8 when real work starts.

  SBUF/partition: w1 64KB + w2 64KB + hs 32KB + x 16KB + y 8KB ~= 184KB.
  PSUM banks: 3 (phase-1 psum_h) + 4 (y tiles) + 1 (pre-warm scratch) = 8.
"""

import numpy as np
import ml_dtypes

import concourse.bacc as bacc
import concourse.mybir as mybir
import concourse.tile as tile
from concourse.bass_utils import run_bass_kernel_spmd

P = 128
H = 1024
F = 4096
E = 8
N_CORES = 8
CHUNK = 512
HT = H // P   # 8
FT = F // P   # 32

BF16 = mybir.dt.bfloat16
F32 = mybir.dt.float32

# Results of the most recent device run (BassKernelResults); lets a test
# harness read exec_time_ns / trace paths without changing kernel()'s API.
LAST_RESULTS = None

_program_cache = {}

# "silu": single ACT op (hardware). "sigmoid_mul": sigmoid + DVE multiply —
# only used for CoreSim validation (the simulator doesn't implement Silu).
SILU_MODE = "silu"


def _build_program(C):
    """Build the per-core Bass program for capacity C (multiple of 128)."""
    assert C % P == 0
    nc = bacc.Bacc(None, name="expert_mlp")

    xt_d = nc.dram_tensor("xt", (HT, P, C), BF16, kind="ExternalInput")
    w1t_d = nc.dram_tensor("w1t", (HT, P, F), BF16, kind="ExternalInput")
    w2t_d = nc.dram_tensor("w2t", (FT, P, H), BF16, kind="ExternalInput")
    wc_d = nc.dram_tensor("wc", (P, C // P), F32, kind="ExternalInput")
    y_d = nc.dram_tensor("y", (C, H), F32, kind="ExternalOutput")

    silu = mybir.ActivationFunctionType.Silu

    with tile.TileContext(nc) as tc:
        with (
            tc.tile_pool(name="wpool", bufs=1) as wpool,
            tc.tile_pool(name="xpool", bufs=2) as xpool,
            tc.tile_pool(name="hpool", bufs=1) as hpool,
            tc.tile_pool(name="ypool", bufs=4) as ypool,
            tc.tile_pool(name="spool", bufs=2) as spool,
            tc.tile_pool(name="psh", bufs=3, space="PSUM") as psh,
            tc.tile_pool(name="psy", bufs=1, space="PSUM") as psy,
        ):
            # Weight / x loads are emitted in CONSUMPTION order so the DMA
            # queue FIFOs deliver bytes just ahead of the PE: w1 column
            # group g feeds phase-1 iters 4g..4g+3, and w2[f] feeds pass A
            # at iter f+2, so they are interleaved; chunk-0 x gates the very
            # first matmul and goes right after w1's first column group.
            # Later chunks' x loads are prefetched one chunk ahead.
            W1_COL_GROUPS = 8
            w1_sb = [
                wpool.tile([P, F], BF16, tag=f"w1_{h}", name=f"w1_{h}")
                for h in range(HT)
            ]
            w2_sb = [
                wpool.tile([P, H], BF16, tag=f"w2_{f}", name=f"w2_{f}")
                for f in range(FT)
            ]
            cols = F // W1_COL_GROUPS
            f_per_g = FT // W1_COL_GROUPS

            def load_x_chunk(c0, cw):
                x_sb = []
                for h in range(HT):
                    t = xpool.tile([P, CHUNK], BF16, tag=f"x_{h}", name=f"x_{h}")[:, :cw]
                    nc.sync.dma_start(t[:], xt_d[h, :, c0 : c0 + cw])
                    x_sb.append(t)
                return x_sb

            # PE pre-warm: dependency-free matmuls on a zeroed scratch tile
            # run while the first weight/x DMAs are in flight, so the HAM
            # clock gate is already at 8/8 when the real matmuls start.
            warm_sb = spool.tile([P, P], BF16, tag="warm", name="warm_sb")
            nc.gpsimd.memset(warm_sb[:], 0.0)
            warm_ps = psy.tile([P, 64], F32, tag="warm_ps", name="warm_ps")
            for _ in range(120):
                nc.tensor.matmul(warm_ps[:], warm_sb[:], warm_sb[:, :64])

            n_chunks = (C + CHUNK - 1) // CHUNK
            x_chunks = []

            def load_w1_group(g):
                for h in range(HT):
                    nc.sync.dma_start(
                        w1_sb[h][:, g * cols : (g + 1) * cols],
                        w1t_d[h, :, g * cols : (g + 1) * cols],
                    )

            # Exact demand-order interleave: w1 group k feeds phase-1 iters
            # 4k..4k+3, w2[f] feeds pass A at iter f+2, so group k is
            # emitted just before w2[4k-2].
            load_w1_group(0)
            x_chunks.append(load_x_chunk(0, min(CHUNK, C)))
            nc.sync.dma_start(w2_sb[0][:], w2t_d[0])
            nc.sync.dma_start(w2_sb[1][:], w2t_d[1])
            for g in range(1, W1_COL_GROUPS):
                load_w1_group(g)
                for f in range(4 * g - 2, 4 * g + 2):
                    nc.sync.dma_start(w2_sb[f][:], w2t_d[f])
            nc.sync.dma_start(w2_sb[FT - 2][:], w2t_d[FT - 2])
            nc.sync.dma_start(w2_sb[FT - 1][:], w2t_d[FT - 1])

            wc_sb = wpool.tile([P, C // P], F32, tag="wc", name="wc_sb")
            nc.sync.dma_start(wc_sb[:], wc_d[:])

            if n_chunks > 1:
                x_chunks.append(load_x_chunk(CHUNK, min(CHUNK, C - CHUNK)))

            for ci in range(n_chunks):
                c0 = ci * CHUNK
                cw = min(CHUNK, C - c0)
                ctiles = cw // P

                if ci + 2 < n_chunks:
                    cp = (ci + 2) * CHUNK
                    x_chunks.append(load_x_chunk(cp, min(CHUNK, C - cp)))
                x_sb = x_chunks[ci]

                hs = hpool.tile([P, FT, CHUNK], BF16, tag="hs", name="hs")[:, :, :cw]
                py = [psy.tile([P, 512], F32, tag=f"py_{t}", name=f"py_{t}") for t in range(ctiles)]

                # phase 1 (h -> silu -> hs) software-pipelined with pass A
                # (first H-half of y), shifted by two f so the PE never
                # waits on the ACT engine's silu.
                SHIFT = 1
                for f in range(FT + SHIFT):
                    if f < FT:
                        ph = psh.tile([P, CHUNK], F32, tag="ph", name="ph")[:, :cw]
                        for h in range(HT):
                            nc.tensor.matmul(
                                ph[:],
                                w1_sb[h][:, f * P : (f + 1) * P],
                                x_sb[h][:],
                                start=(h == 0),
                                stop=(h == HT - 1),
                            )
                        if SILU_MODE == "silu":
                            nc.scalar.activation(hs[:, f, :], ph[:], silu)
                        else:
                            sg = spool.tile([P, CHUNK], F32, tag="sg", name="sg")[:, :cw]
                            nc.scalar.activation(
                                sg[:], ph[:], mybir.ActivationFunctionType.Sigmoid
                            )
                            nc.vector.tensor_mul(hs[:, f, :], sg[:], ph[:])
                    if f >= SHIFT:
                        fp = f - SHIFT
                        for t in range(ctiles):
                            nc.tensor.matmul(
                                py[t][:],
                                hs[:, fp, t * P : (t + 1) * P],
                                w2_sb[fp][:, 0:512],
                                start=(fp == 0),
                                stop=(fp == FT - 1),
                            )
                for t in range(ctiles):
                    yh = ypool.tile([P, 512], F32, tag="yh", name="yh")
                    nc.vector.tensor_scalar_mul(
                        yh[:], py[t][:], wc_sb[:, c0 // P + t : c0 // P + t + 1]
                    )
                    for q in range(4):
                        nc.sync.dma_start(
                            y_d[c0 + t * P : c0 + (t + 1) * P, q * 128 : (q + 1) * 128],
                            yh[:, q * 128 : (q + 1) * 128],
                        )

                # pass B: second H-half of y, re-reading hs. t-outer so each
                # y tile's scale + DMA-out overlaps the remaining matmuls
                # (keeps the kernel tail short).
                py = [psy.tile([P, 512], F32, tag=f"py_{t}", name=f"py_{t}") for t in range(ctiles)]
                for t in range(ctiles):
                    for f in range(FT):
                        nc.tensor.matmul(
                            py[t][:],
                            hs[:, f, t * P : (t + 1) * P],
                            w2_sb[f][:, 512:1024],
                            start=(f == 0),
                            stop=(f == FT - 1),
                        )
                    yh = ypool.tile([P, 512], F32, tag="yh", name="yh")
                    nc.vector.tensor_scalar_mul(
                        yh[:], py[t][:], wc_sb[:, c0 // P + t : c0 // P + t + 1]
                    )
                    for q in range(4):
                        nc.sync.dma_start(
                            y_d[
                                c0 + t * P : c0 + (t + 1) * P,
                                512 + q * 128 : 512 + (q + 1) * 128,
                            ],
                            yh[:, q * 128 : (q + 1) * 128],
                        )

    nc.compile()
    return nc


def _get_program(C):
    if C not in _program_cache:
        _program_cache[C] = _build_program(C)
    return _program_cache[C]


def _route(topk_e, topk_w):
    """Per-expert token indices and combined gate weights (duplicate top-k
    hits of the same expert are merged by summing their weights, matching
    the reference's repeated +=)."""
    idxs, wts = [], []
    for e in range(E):
        m = topk_e == e
        idx = np.nonzero(m.any(axis=1))[0]
        we = (topk_w.astype(np.float32) * m).sum(axis=1)[idx]
        idxs.append(idx)
        wts.append(we)
    return idxs, wts


def _ensure_device_healthy():
    """Probe the accelerator; if wedged (NRT unrecoverable), axon_reset it.
    Best-effort: silently skips when not running under the axon proxy."""
    try:
        import jax
        import jax.numpy as jnp
    except Exception:
        return
    for _ in range(3):
        try:
            a = jnp.ones((8, 8))
            float((a @ a).sum())
            return
        except Exception:
            try:
                import ctypes

                lib = ctypes.CDLL("/opt/axon/libaxon_pjrt.so")
                lib.axon_reset.restype = ctypes.c_int64
                lib.axon_reset()
            except Exception:
                return


def kernel(x, topk_e, topk_w, w1, w2):
    global LAST_RESULTS
    _ensure_device_healthy()
    x = np.ascontiguousarray(np.asarray(x), dtype=np.float32)
    topk_e = np.asarray(topk_e)
    topk_w = np.asarray(topk_w)
    w1 = np.asarray(w1, dtype=np.float32)
    w2 = np.asarray(w2, dtype=np.float32)
    S = x.shape[0]

    idxs, wts = _route(topk_e, topk_w)
    cmax = max(len(i) for i in idxs)
    C = max(P, -(-cmax // P) * P)

    nc = _get_program(C)

    bf = ml_dtypes.bfloat16
    in_maps = []
    for e in range(E):
        idx = idxs[e]
        n = len(idx)
        xe = np.zeros((C, H), np.float32)
        xe[:n] = x[idx]
        xt = np.ascontiguousarray(xe.T).astype(bf).reshape(HT, P, C)
        w1t = np.ascontiguousarray(w1[e].T).astype(bf).reshape(HT, P, F)
        w2t = np.ascontiguousarray(w2[e].T).astype(bf).reshape(FT, P, H)
        wc = np.zeros((C,), np.float32)
        wc[:n] = wts[e]
        wc = np.ascontiguousarray(wc.reshape(C // P, P).T)
        in_maps.append({"xt": xt, "w1t": w1t, "w2t": w2t, "wc": wc})

    res = run_bass_kernel_spmd(nc, in_maps, core_ids=list(range(N_CORES)))
    LAST_RESULTS = res

    y = np.zeros((S, H), np.float32)
    for e in range(E):
        idx = idxs[e]
        y[idx] += res.results[e]["y"][: len(idx)]
    return y


# revision 32
# speedup vs baseline: 1.0039x; 1.0039x over previous
"""Trainium2 Bass kernel for nn_ExpertMLP (MoE routing).

Strategy (expert-parallel, host-side dispatch):
  - E == n_cores == 8: core e owns expert e's weights.
  - Host computes the routing (which tokens hit expert e, with combined
    gate weight summed over duplicate top-k hits), gathers those tokens
    into a padded [C, H] buffer per expert, and ships core e:
        xt  = gathered tokens, transposed  [H, C]   (bf16)
        w1t = w1[e].T                      [H, F]   (bf16)
        w2t = w2[e].T                      [F, H]   (bf16)
        wc  = combined gate weights        [C]      (f32)
  - Device computes  y_e = (silu(x_e @ w1[e].T) @ w2[e].T) * wc[:, None]
    with all matmuls in bf16 (f32 PSUM accumulation).
  - Host scatter-adds per-expert outputs back into the full [S, H] output.

Device kernel layout (per core):
  Phase 1 computes h in F-major layout (hT [F, C]) so that phase 2 can use
  hT tiles directly as the stationary matmul operand -- no transposes
  anywhere on device (host provides x/w1/w2 pre-transposed).

  Per 512-token chunk:
    phase 1:  for f in 32:  psum_h[128, cw] += w1t[h, f-tile].T @ xt[h]   (8 MMs)
              silu(psum_h) -> hs[:, f, :]  (bf16, ACT engine)
    pass A (interleaved, shifted by two f): y[:, 0:512] accumulated over f
              psum_yA[t] += hs[f, t-tile].T @ w2t[f][:, 0:512]
    pass B:   y[:, 512:1024] accumulated over f (re-reads hs), t-outer
    epilogue: y_sbuf = psum_y * wc  (per-partition scalar), DMA out 4-way.

  DMAs are emitted in consumption order (w1 column groups interleaved with
  the w2 tiles used alongside them; x prefetched one chunk ahead) and the
  PE is pre-warmed with dependency-free matmuls so the HAM clock gate is
  at 8/8 when real work starts.

  SBUF/partition: w1 64KB + w2 64KB + hs 32KB + x 16KB + y 8KB ~= 184KB.
  PSUM banks: 3 (phase-1 psum_h) + 4 (y tiles) + 1 (pre-warm scratch) = 8.
"""

import numpy as np
import ml_dtypes

import concourse.bacc as bacc
import concourse.mybir as mybir
import concourse.tile as tile
from concourse.bass_utils import run_bass_kernel_spmd

P = 128
H = 1024
F = 4096
E = 8
N_CORES = 8
CHUNK = 512
HT = H // P   # 8
FT = F // P   # 32

BF16 = mybir.dt.bfloat16
F32 = mybir.dt.float32

# Results of the most recent device run (BassKernelResults); lets a test
# harness read exec_time_ns / trace paths without changing kernel()'s API.
LAST_RESULTS = None

_program_cache = {}

# "silu": single ACT op (hardware). "sigmoid_mul": sigmoid + DVE multiply —
# only used for CoreSim validation (the simulator doesn't implement Silu).
SILU_MODE = "silu"


def _build_program(C):
    """Build the per-core Bass program for capacity C (multiple of 128)."""
    assert C % P == 0
    nc = bacc.Bacc(None, name="expert_mlp")

    xt_d = nc.dram_tensor("xt", (HT, P, C), BF16, kind="ExternalInput")
    w1t_d = nc.dram_tensor("w1t", (HT, P, F), BF16, kind="ExternalInput")
    w2t_d = nc.dram_tensor("w2t", (FT, P, H), BF16, kind="ExternalInput")
    wc_d = nc.dram_tensor("wc", (P, C // P), F32, kind="ExternalInput")
    y_d = nc.dram_tensor("y", (C, H), F32, kind="ExternalOutput")

    silu = mybir.ActivationFunctionType.Silu

    with tile.TileContext(nc) as tc:
        with (
            tc.tile_pool(name="wpool", bufs=1) as wpool,
            tc.tile_pool(name="xpool", bufs=2) as xpool,
            tc.tile_pool(name="hpool", bufs=1) as hpool,
            tc.tile_pool(name="ypool", bufs=4) as ypool,
            tc.tile_pool(name="spool", bufs=2) as spool,
            tc.tile_pool(name="psh", bufs=3, space="PSUM") as psh,
            tc.tile_pool(name="psy", bufs=1, space="PSUM") as psy,
        ):
            # Weight / x loads are emitted in CONSUMPTION order so the DMA
            # queue FIFOs deliver bytes just ahead of the PE: w1 column
            # group g feeds phase-1 iters 4g..4g+3, and w2[f] feeds pass A
            # at iter f+2, so they are interleaved; chunk-0 x gates the very
            # first matmul and goes right after w1's first column group.
            # Later chunks' x loads are prefetched one chunk ahead.
            W1_COL_GROUPS = 8
            w1_sb = [
                wpool.tile([P, F], BF16, tag=f"w1_{h}", name=f"w1_{h}")
                for h in range(HT)
            ]
            w2_sb = [
                wpool.tile([P, H], BF16, tag=f"w2_{f}", name=f"w2_{f}")
                for f in range(FT)
            ]
            cols = F // W1_COL_GROUPS
            f_per_g = FT // W1_COL_GROUPS

            def load_x_chunk(c0, cw):
                x_sb = []
                for h in range(HT):
                    t = xpool.tile([P, CHUNK], BF16, tag=f"x_{h}", name=f"x_{h}")[:, :cw]
                    nc.sync.dma_start(t[:], xt_d[h, :, c0 : c0 + cw])
                    x_sb.append(t)
                return x_sb

            # PE pre-warm: dependency-free matmuls on a zeroed scratch tile
            # run while the first weight/x DMAs are in flight, so the HAM
            # clock gate is already at 8/8 when the real matmuls start.
            warm_sb = spool.tile([P, P], BF16, tag="warm", name="warm_sb")
            nc.gpsimd.memset(warm_sb[:], 0.0)
            warm_ps = psy.tile([P, 64], F32, tag="warm_ps", name="warm_ps")
            for _ in range(120):
                nc.tensor.matmul(warm_ps[:], warm_sb[:], warm_sb[:, :64])

            n_chunks = (C + CHUNK - 1) // CHUNK
            x_chunks = []

            def load_w1_group(g):
                for h in range(HT):
                    nc.sync.dma_start(
                        w1_sb[h][:, g * cols : (g + 1) * cols],
                        w1t_d[h, :, g * cols : (g + 1) * cols],
                    )

            # Exact demand-order interleave: w1 group k feeds phase-1 iters
            # 4k..4k+3, w2[f] feeds pass A at iter f+2, so group k is
            # emitted just before w2[4k-2].
            load_w1_group(0)
            x_chunks.append(load_x_chunk(0, min(CHUNK, C)))
            nc.sync.dma_start(w2_sb[0][:], w2t_d[0])
            nc.sync.dma_start(w2_sb[1][:], w2t_d[1])
            for g in range(1, W1_COL_GROUPS):
                load_w1_group(g)
                for f in range(4 * g - 2, 4 * g + 2):
                    nc.sync.dma_start(w2_sb[f][:], w2t_d[f])
            nc.sync.dma_start(w2_sb[FT - 2][:], w2t_d[FT - 2])
            nc.sync.dma_start(w2_sb[FT - 1][:], w2t_d[FT - 1])

            wc_sb = wpool.tile([P, C // P], F32, tag="wc", name="wc_sb")
            nc.sync.dma_start(wc_sb[:], wc_d[:])

            if n_chunks > 1:
                x_chunks.append(load_x_chunk(CHUNK, min(CHUNK, C - CHUNK)))

            for ci in range(n_chunks):
                c0 = ci * CHUNK
                cw = min(CHUNK, C - c0)
                ctiles = cw // P

                if ci + 2 < n_chunks:
                    cp = (ci + 2) * CHUNK
                    x_chunks.append(load_x_chunk(cp, min(CHUNK, C - cp)))
                x_sb = x_chunks[ci]

                hs = hpool.tile([P, FT, CHUNK], BF16, tag="hs", name="hs")[:, :, :cw]
                py = [psy.tile([P, 512], F32, tag=f"py_{t}", name=f"py_{t}") for t in range(ctiles)]

                # phase 1 (h -> silu -> hs) software-pipelined with pass A
                # (first H-half of y), shifted by two f so the PE never
                # waits on the ACT engine's silu.
                SHIFT = 2
                for f in range(FT + SHIFT):
                    if f < FT:
                        ph = psh.tile([P, CHUNK], F32, tag="ph", name="ph")[:, :cw]
                        for h in range(HT):
                            nc.tensor.matmul(
                                ph[:],
                                w1_sb[h][:, f * P : (f + 1) * P],
                                x_sb[h][:],
                                start=(h == 0),
                                stop=(h == HT - 1),
                            )
                        if SILU_MODE == "silu":
                            nc.scalar.activation(hs[:, f, :], ph[:], silu)
                        else:
                            sg = spool.tile([P, CHUNK], F32, tag="sg", name="sg")[:, :cw]
                            nc.scalar.activation(
                                sg[:], ph[:], mybir.ActivationFunctionType.Sigmoid
                            )
                            nc.vector.tensor_mul(hs[:, f, :], sg[:], ph[:])
                    if f >= SHIFT:
                        fp = f - SHIFT
                        for t in range(ctiles):
                            nc.tensor.matmul(
                                py[t][:],
                                hs[:, fp, t * P : (t + 1) * P],
                                w2_sb[fp][:, 0:512],
                                start=(fp == 0),
                                stop=(fp == FT - 1),
                            )
                for t in range(ctiles):
                    yh = ypool.tile([P, 512], F32, tag="yh", name="yh")
                    nc.vector.tensor_scalar_mul(
                        yh[:], py[t][:], wc_sb[:, c0 // P + t : c0 // P + t + 1]
                    )
                    for q in range(4):
                        nc.sync.dma_start(
                            y_d[c0 + t * P : c0 + (t + 1) * P, q * 128 : (q + 1) * 128],
                            yh[:, q * 128 : (q + 1) * 128],
                        )

                # pass B: second H-half of y, re-reading hs. t-outer so each
                # y tile's scale + DMA-out overlaps the remaining matmuls
                # (keeps the kernel tail short).
                py = [psy.tile([P, 512], F32, tag=f"py_{t}", name=f"py_{t}") for t in range(ctiles)]
                for t in range(ctiles):
                    for f in range(FT):
                        nc.tensor.matmul(
                            py[t][:],
                            hs[:, f, t * P : (t + 1) * P],
                            w2_sb[f][:, 512:1024],
                            start=(f == 0),
                            stop=(f == FT - 1),
                        )
                    yh = ypool.tile([P, 512], F32, tag="yh", name="yh")
                    nc.vector.tensor_scalar_mul(
                        yh[:], py[t][:], wc_sb[:, c0 // P + t : c0 // P + t + 1]
                    )
                    for q in range(4):
                        nc.sync.dma_start(
                            y_d[
                                c0 + t * P : c0 + (t + 1) * P,
                                512 + q * 128 : 512 + (q + 1) * 128,
                            ],
                            yh[:, q * 128 : (q + 1) * 128],
                        )

    nc.compile()
    return nc


def _get_program(C):
    if C not in _program_cache:
        _program_cache[C] = _build_program(C)
    return _program_cache[C]


def _route(topk_e, topk_w):
    """Per-expert token indices and combined gate weights (duplicate top-k
    hits of the same expert are merged by summing their weights, matching
    the reference's repeated +=)."""
    idxs, wts = [], []
    for e in range(E):
        m = topk_e == e
        idx = np.nonzero(m.any(axis=1))[0]
        we = (topk_w.astype(np.float32) * m).sum(axis=1)[idx]
        idxs.append(idx)
        wts.append(we)
    return idxs, wts


def _ensure_device_healthy():
    """Probe the accelerator; if wedged (NRT unrecoverable), axon_reset it.
    Best-effort: silently skips when not running under the axon proxy."""
    try:
        import jax
        import jax.numpy as jnp
    except Exception:
        return
    for _ in range(3):
        try:
            a = jnp.ones((8, 8))
            float((a @ a).sum())
            return
        except Exception:
            try:
                import ctypes

                lib = ctypes.CDLL("/opt/axon/libaxon_pjrt.so")
                lib.axon_reset.restype = ctypes.c_int64
                lib.axon_reset()
            except Exception:
                return


def kernel(x, topk_e, topk_w, w1, w2):
    global LAST_RESULTS
    _ensure_device_healthy()
    x = np.ascontiguousarray(np.asarray(x), dtype=np.float32)
    topk_e = np.asarray(topk_e)
    topk_w = np.asarray(topk_w)
    w1 = np.asarray(w1, dtype=np.float32)
    w2 = np.asarray(w2, dtype=np.float32)
    S = x.shape[0]

    idxs, wts = _route(topk_e, topk_w)
    cmax = max(len(i) for i in idxs)
    C = max(P, -(-cmax // P) * P)

    nc = _get_program(C)

    bf = ml_dtypes.bfloat16
    in_maps = []
    for e in range(E):
        idx = idxs[e]
        n = len(idx)
        xe = np.zeros((C, H), np.float32)
        xe[:n] = x[idx]
        xt = np.ascontiguousarray(xe.T).astype(bf).reshape(HT, P, C)
        w1t = np.ascontiguousarray(w1[e].T).astype(bf).reshape(HT, P, F)
        w2t = np.ascontiguousarray(w2[e].T).astype(bf).reshape(FT, P, H)
        wc = np.zeros((C,), np.float32)
        wc[:n] = wts[e]
        wc = np.ascontiguousarray(wc.reshape(C // P, P).T)
        in_maps.append({"xt": xt, "w1t": w1t, "w2t": w2t, "wc": wc})

    res = run_bass_kernel_spmd(nc, in_maps, core_ids=list(range(N_CORES)))
    LAST_RESULTS = res

    y = np.zeros((S, H), np.float32)
    for e in range(E):
        idx = idxs[e]
        y[idx] += res.results[e]["y"][: len(idx)]
    return y
